# revision 37
# baseline (speedup 1.0000x reference)
"""Trainium2 Bass kernel for nn_BPPSModel (type-routed atom MLP + segment pooling).

Strategy (v4): v3 plus engine rebalance — relu(z1) split across scalar and
vector engines, Square fully on scalar, relu(z2) fully on vector; the v
reduction is one fp8 DoubleRow matmul; e/v ship straight from PSUM with one
DMA per tile pair; xs tiles stream with one DMA per tile pair.

Strategy (v3):
- Atoms sharded contiguously across 8 cores (50000 each). The host sorts each
  core's atoms by type, pads each type run to a 512 multiple, and lays the
  features out transposed [128 part, 4 kchunk, A_PAD atoms] in bf16 during the
  fp32->bf16 conversion pass. The device then streams tiles with plain
  sequential DMA - no gpsimd gather.
- LayerNorm folding (host): W1/W2 centered over their output dim absorbs the
  mean subtraction; LN scale-invariance (g=1, b=0) lets layer-1's inverse
  sigma cancel inside layer-2's LN; layer-2's inverse sigma is applied on the
  host from a device-computed sum-of-squares v = sum(z2c^2).
- Everything runs in plain bf16 (tolerance 2e-2; measured ~1.1e-3): per tile
  of 512 atoms, 8 L1 matmuls, relu (scalar), 4 L2 matmuls, relu (scalar),
  square (vector), then M=1 reduce-matmuls for e' = wout.relu(z2c) and
  v = ones.z2c^2. Host applies e = e' * rsqrt(v/256 + eps) and pools with
  per-structure bincounts summed across cores.
- Custom PJRT runner: inputs are built directly in the concatenated global
  layout run_bass_via_pjrt would otherwise np.concatenate per call (saves a
  410MB host copy per call).
"""

import numpy as np
import ml_dtypes

N_ATOMS = 400000
N_FEAT = 512
H1 = 256
H2 = 256
N_TYPES = 4
NUM_STRUCTS = 4096
LN_EPS = 1e-5
N_CORES = 8
ATOMS_PER_CORE = N_ATOMS // N_CORES
TILE_A = 512  # atoms per tile (free dim)
KF = N_FEAT // 128  # 4
K2 = H1 // 128  # 2
O1 = H1 // 128  # 2
O2 = H2 // 128  # 2

_module_cache = {}
_runner_cache = {}


def _numpy_reference(features, W1, W2, Wout, g1, b1, g2, b2, comp_w, numbers, batch):
    x = features.astype(np.float32)
    t = numbers.astype(np.int64)

    def linmap(h, W):
        out = np.zeros((h.shape[0], W.shape[2]), dtype=np.float32)
        for ty in range(W.shape[0]):
            m = t == ty
            out[m] = h[m] @ W[ty]
        return out

    def ln(h, g, b):
        mu = h.mean(axis=-1, keepdims=True)
        var = h.var(axis=-1, keepdims=True)
        return (h - mu) / np.sqrt(var + LN_EPS) * g + b

    h = np.maximum(ln(linmap(x, W1), g1, b1), 0.0)
    h = np.maximum(ln(linmap(h, W2), g2, b2), 0.0)
    atom_e = linmap(h, Wout)[:, 0]
    energies = np.bincount(batch.astype(np.int64), weights=atom_e, minlength=NUM_STRUCTS)
    onehot_w = comp_w[0].astype(np.float64)[t]
    comp = np.bincount(batch.astype(np.int64), weights=onehot_w, minlength=NUM_STRUCTS)
    return (energies + comp).reshape(NUM_STRUCTS, 1).astype(np.float32)


def _build_schedule(numbers):
    """Common type-tile schedule + per-core sorted atom permutation.

    Returns (tile_t, per_core) where tile_t[j] is the type of tile j (all
    tiles TILE_A wide) and per_core[c] has perm (padded global atom ids,
    [T*TILE_A]) and valid ([T*TILE_A] bool). T is forced even so the device
    can batch DMAs over tile pairs."""
    numbers = numbers.astype(np.int64)
    counts = np.zeros((N_CORES, N_TYPES), dtype=np.int64)
    orders = []
    for c in range(N_CORES):
        nb = numbers[c * ATOMS_PER_CORE : (c + 1) * ATOMS_PER_CORE]
        orders.append(np.argsort(nb, kind="stable"))
        counts[c] = np.bincount(nb, minlength=N_TYPES)
    tiles_per_type = [
        int(np.ceil(counts[:, t].max() / TILE_A)) for t in range(N_TYPES)
    ]
    if sum(tiles_per_type) % 2:
        tiles_per_type[-1] += 1
    tile_t = []
    for t in range(N_TYPES):
        tile_t.extend([t] * tiles_per_type[t])
    T = len(tile_t)
    A_PAD = T * TILE_A

    per_core = []
    for c in range(N_CORES):
        perm = np.zeros(A_PAD, dtype=np.int64)
        valid = np.zeros(A_PAD, dtype=bool)
        base = c * ATOMS_PER_CORE
        off = 0  # within this core's sorted order
        pos = 0  # within the padded layout
        for t in range(N_TYPES):
            cnt = int(counts[c, t])
            run = base + orders[c][off : off + cnt]
            off += cnt
            width = tiles_per_type[t] * TILE_A
            perm[pos : pos + cnt] = run
            valid[pos : pos + cnt] = True
            if cnt < width:
                # padding lanes: repeat a real atom id so gathered data is
                # defined; masked out on the host afterwards
                perm[pos + cnt : pos + width] = run[-1] if cnt else base
            pos += width
        per_core.append(dict(perm=perm, valid=valid))
    return tile_t, per_core


def _build_module(tile_t):
    import concourse.tile as tile
    from concourse import bacc, mybir

    F32 = mybir.dt.float32
    BF16 = mybir.dt.bfloat16
    F8 = mybir.dt.float8e4
    AF = mybir.ActivationFunctionType
    DR = mybir.MatmulPerfMode.DoubleRow

    T = len(tile_t)
    assert T % 2 == 0
    A_PAD = T * TILE_A
    nc = bacc.Bacc(
        "TRN2", target_bir_lowering=False, debug=False, num_devices=N_CORES,
        enable_asserts=False,
    )
    xs_in = nc.dram_tensor("xs", [128, KF, A_PAD], F8, kind="ExternalInput")
    w1h_in = nc.dram_tensor("w1h", [N_TYPES, N_FEAT, H1], F8, kind="ExternalInput")
    w2h_in = nc.dram_tensor("w2h", [N_TYPES, H1, H2], BF16, kind="ExternalInput")
    # wout pre-replicated 16x on the host so the e matmul has the same PE
    # tile size as the DR v matmul (they co-issue in different column tiles)
    woh_in = nc.dram_tensor("wo_h", [N_TYPES, H2, 16], BF16, kind="ExternalInput")
    ones_in = nc.dram_tensor("ones8", [128, 32], F8, kind="ExternalInput")
    # [pair, row(v=p0 / e=p32), slot(tile within pair), atoms]
    ev_out = nc.dram_tensor("ev_out", [T // 2, 2, 2, TILE_A], F32, kind="ExternalOutput")

    # Act/DVE split point of the relu(z1) pass, in atoms (of TILE_A). The
    # scalar engine also runs the full Square pass; the vector engine the
    # full relu(z2) pass + the ev staging copy; 464/48 balances measured
    # engine rates (Act 0.83 ns/elem + 255 ns/instr, DVE 1.04 + 150).
    R1_SPLIT = 464

    with tile.TileContext(nc) as tc:
        with (
            tc.tile_pool(name="const", bufs=1) as cp,
            tc.tile_pool(name="work", bufs=5) as wp,
            tc.tile_pool(name="gat", bufs=3) as gp,
            tc.tile_pool(name="ps1", bufs=1, space="PSUM") as ps1,
            tc.tile_pool(name="ps2", bufs=3, space="PSUM") as ps2,
        ):
            w1h = cp.tile([128, N_TYPES, KF, O1, 128], F8)
            nc.sync.dma_start(
                w1h[:], w1h_in.ap().rearrange("t (k p) (o q) -> p t k o q", p=128, q=128)
            )
            w2h = cp.tile([128, N_TYPES, K2, O2, 128], BF16)
            nc.sync.dma_start(
                w2h[:], w2h_in.ap().rearrange("t (k p) (o q) -> p t k o q", p=128, q=128)
            )
            wofh = cp.tile([128, N_TYPES, K2, 16], BF16)
            nc.sync.dma_start(
                wofh[:], woh_in.ap().rearrange("t (k p) m -> p t k m", p=128)
            )
            # dual-fp8 ldweights requires stationary free size >= 16: use 16
            # identical ones columns; all 16 output partitions get the same v
            ones8 = cp.tile([128, 2, 16], F8)
            nc.sync.dma_start(ones8[:], ones_in.ap().rearrange("p (k m) -> p k m", k=2))

            # ---- software-pipelined loop ------------------------------------
            # stage lags (iteration i): L1(i) | r1(i-1) | L2(i-2), r2/sq(i-2)
            # | v/e(i-4), ev-copy(i-4) | ev-DMA(pair). Every cross-engine
            # input is produced at least one iteration earlier (or early
            # enough in this iteration), so no engine busy-waits at its queue
            # head. PSUM: one z1 buffer (2 banks) + three z2 buffers (6
            # banks); the triple-buffered z2 gives the e/v/copy tail a full
            # iteration of slack before its bank is rewritten.
            ghp = {}     # pair -> gather tile
            z1_t, r1_t, z2_t, r2_t, sq_t = {}, {}, {}, {}, {}
            evp = {}     # pair -> ev staging tile

            def prefetch(p):
                g = gp.tile([128, KF, 2 * TILE_A], F8, tag="gh")
                nc.sync.dma_start(
                    g[:], xs_in.ap()[:, :, 2 * p * TILE_A : (2 * p + 2) * TILE_A]
                )
                ghp[p] = g

            prefetch(0)
            if T > 2:
                prefetch(1)
            for i in range(T + 5):
                # --- PE: L2 of tile i-2 (its r1 finished last iteration) ----
                if 0 <= i - 2 < T:
                    j = i - 2
                    z2 = ps2.tile([128, O2, TILE_A], F32, tag="z2")
                    z2_t[j] = z2
                    r1 = r1_t[j]
                    for o in range(O2):
                        for k in range(K2):
                            nc.tensor.matmul(
                                z2[:, o], w2h[:, tile_t[j], k, o], r1[:, k],
                                start=(k == 0), stop=(k == K2 - 1),
                            )
                # --- PE: e and v reductions of tile i-4 ---------------------
                # (e: bf16 M=1 at PE column tile 32; v: fp8 M=1 at column
                # tile 0; interleaved k so each e matmul co-issues with the
                # v matmul of the same k in the other column tile)
                if 0 <= i - 4 < T:
                    j = i - 4
                    z2 = z2_t[j]
                    for k in range(K2):
                        nc.tensor.matmul(
                            z2[32:33, 0], wofh[:, tile_t[j], k, 0:1],
                            r2_t[j][:, k],
                            start=(k == 0), stop=(k == K2 - 1),
                            tile_position=(0, 32),
                        )
                        nc.tensor.matmul(
                            z2[0:1, 0], ones8[:, k, 0:1], sq_t[j][:, k],
                            start=(k == 0), stop=(k == K2 - 1),
                            tile_position=(0, 0),
                        )
                # --- PE: L1 of tile i (fp8 DoubleRow over k-chunk pairs) ----
                # (last on the PE so the single z1 buffer's previous tile has
                # been consumed by r1 well before this writes it)
                if i < T:
                    j = i
                    gh = ghp[j // 2][:, :, (j % 2) * TILE_A : (j % 2 + 1) * TILE_A]
                    z1 = ps1.tile([128, O1, TILE_A], F32, tag="z1")
                    z1_t[j] = z1
                    for o in range(O1):
                        for k in range(0, KF, 2):
                            nc.tensor.matmul(
                                z1[:, o], w1h[:, tile_t[j], k : k + 2, o],
                                gh[:, k : k + 2],
                                start=(k == 0), stop=(k == KF - 2),
                                perf_mode=DR,
                            )
                    if j % 2 == 0 and j + 4 < T:
                        prefetch(j // 2 + 2)
                # --- Act: relu(z1) head of tile i-1, then Square of i-2 -----
                if 0 <= i - 1 < T:
                    j = i - 1
                    r1 = wp.tile([128, O1, TILE_A], BF16, tag="r1")
                    r1_t[j] = r1
                    nc.scalar.activation(
                        r1[:, :, :R1_SPLIT], z1_t[j][:, :, :R1_SPLIT], AF.Relu
                    )
                if 0 <= i - 2 < T:
                    j = i - 2
                    sq = wp.tile([128, O2, TILE_A], F8, tag="sq")
                    sq_t[j] = sq
                    nc.scalar.activation(sq[:], z2_t[j][:], AF.Square)
                # --- DVE: relu(z1) tail of i-1, ev copy of i-4, relu(z2) of
                # i-2 (the copy fills the gap while this iteration's L2 runs)
                if 0 <= i - 1 < T:
                    j = i - 1
                    nc.vector.tensor_scalar_max(
                        r1_t[j][:, :, R1_SPLIT:], z1_t[j][:, :, R1_SPLIT:], 0.0
                    )
                    del z1_t[j]
                if 0 <= i - 4 < T:
                    j = i - 4
                    if j % 2 == 0:
                        evp[j // 2] = wp.tile(
                            [33, 2, TILE_A], F32, tag="tmp_ev", name="tmp_ev"
                        )
                    nc.vector.tensor_copy(evp[j // 2][:, j % 2], z2_t[j][0:33, 0])
                    del z2_t[j], r2_t[j], sq_t[j]
                if 0 <= i - 2 < T:
                    j = i - 2
                    r2 = wp.tile([128, O2, TILE_A], BF16, tag="r2")
                    r2_t[j] = r2
                    nc.vector.tensor_scalar_max(r2[:], z2_t[j][:], 0.0)
                    del r1_t[j]
                # --- SP: ship a completed ev pair ---------------------------
                if 0 <= i - 4 < T and (i - 4) % 2 == 1:
                    p = (i - 4) // 2
                    nc.sync.dma_start(ev_out.ap()[p], evp.pop(p)[0:33:32])

    nc.compile()
    return nc


def _get_runner(nc):
    """Build (once per module) a jitted shard_map runner that takes inputs
    already concatenated along axis 0 - the layout run_bass_via_pjrt builds
    with np.concatenate on every call."""
    key = id(nc)
    if key in _runner_cache:
        return _runner_cache[key]

    import jax
    from jax.experimental.shard_map import shard_map
    from jax.sharding import Mesh, PartitionSpec
    from concourse import bass2jax, mybir

    bass2jax.install_neuronx_cc_hook()

    partition_name = nc.partition_id_tensor.name if nc.partition_id_tensor else None
    in_names = []
    out_names = []
    out_avals = []
    out_shapes = []
    for alloc in nc.m.functions[0].allocations:
        if not isinstance(alloc, mybir.MemoryLocationSet):
            continue
        name = alloc.memorylocations[0].name
        if alloc.kind == "ExternalInput":
            if name != partition_name:
                in_names.append(name)
        elif alloc.kind == "ExternalOutput":
            shape = tuple(alloc.tensor_shape)
            dtype = mybir.dt.np(alloc.dtype)
            out_avals.append(jax.core.ShapedArray(shape, dtype))
            out_names.append(name)
            out_shapes.append((shape, dtype))
    n_params = len(in_names)
    n_outs = len(out_names)
    all_in_names = list(in_names) + list(out_names)
    if partition_name is not None:
        all_in_names.append(partition_name)
    donate = tuple(range(n_params, n_params + n_outs))

    def _body(*args):
        operands = list(args)
        if partition_name is not None:
            operands.append(bass2jax.partition_id_tensor())
        outs = bass2jax._bass_exec_p.bind(
            *operands,
            out_avals=tuple(out_avals),
            in_names=tuple(all_in_names),
            out_names=tuple(out_names),
            lowering_input_output_aliases=(),
            sim_require_finite=True,
            sim_require_nnan=True,
            nc=nc,
        )
        return tuple(outs)

    devices = jax.devices()[:N_CORES]
    mesh = Mesh(np.asarray(devices), ("core",))
    in_specs = (PartitionSpec("core"),) * (n_params + n_outs)
    out_specs = (PartitionSpec("core"),) * n_outs
    sharded = jax.jit(
        shard_map(
            _body, mesh=mesh, in_specs=in_specs, out_specs=out_specs,
            check_rep=False,
        ),
        donate_argnums=donate,
        keep_unused=True,
    )
    runner = (sharded, in_names, out_names, out_shapes)
    _runner_cache[key] = runner
    return runner


def _run_global(nc, global_map):
    """Run the SPMD module; global_map maps input name -> globally
    concatenated array [N_CORES*d0, ...]. Returns {name: [N_CORES, d0, ...]}."""
    sharded, in_names, out_names, out_shapes = _get_runner(nc)
    ins = [np.asarray(global_map[name]) for name in in_names]
    zeros = [
        np.zeros((N_CORES * s[0], *s[1:]), dt) for (s, dt) in out_shapes
    ]
    outs = sharded(*ins, *zeros)
    return {
        name: np.asarray(arr).reshape(N_CORES, *shape)
        for name, arr, (shape, _) in zip(out_names, outs, out_shapes)
    }


def _f8_twiddle_bits32(rows_f32):
    """f32 -> float8_e4m3 bits (round-half-up, flush-to-zero subnormals).
    Vectorized uint ops; ~20x faster than ml_dtypes' generic cast loop."""
    u = rows_f32.view(np.uint32)
    s = ((u >> 24) & np.uint32(0x80)).astype(np.uint8)
    # round mantissa 23->3 bits, carry naturally into the exponent
    t = ((u & np.uint32(0x7FFFFFFF)) + np.uint32(0x00080000)) >> 20
    t = t.astype(np.int32)
    t -= 960  # rebias exponent 127 -> 7
    np.clip(t, 0, 127, out=t)
    return s | t.astype(np.uint8)


def _build_xs_global(features, per_core, A_PAD):
    """fp32 features -> fp8e4m3, type-sorted, transposed [128, KF, A] per
    core, all cores stacked -> [N_CORES*128, KF, A_PAD]."""
    xs = np.empty((N_CORES * 128, KF, A_PAD), dtype=np.uint8)
    CS = 1024
    for c in range(N_CORES):
        perm = per_core[c]["perm"]
        view = xs[c * 128 : (c + 1) * 128]
        for a0 in range(0, A_PAD, CS):
            n = min(CS, A_PAD - a0)
            rows = _f8_twiddle_bits32(features[perm[a0 : a0 + n]])
            view[:, :, a0 : a0 + n] = rows.reshape(n, KF, 128).transpose(2, 1, 0)
    return xs.view(ml_dtypes.float8_e4m3)


def _device_run(features, W1, W2, Wout, comp_w, numbers, batch):
    W1c = W1 - W1.mean(axis=2, keepdims=True)
    W2c = W2 - W2.mean(axis=2, keepdims=True)
    w1h = W1c.astype(ml_dtypes.float8_e4m3)
    w2h = W2c.astype(ml_dtypes.bfloat16)
    wo_h = np.ascontiguousarray(
        np.repeat(Wout[:, :, 0:1], 16, axis=2)
    ).astype(ml_dtypes.bfloat16)

    tile_t, per_core = _build_schedule(numbers)
    key = tuple(tile_t)
    if key not in _module_cache:
        _module_cache[key] = _build_module(tile_t)
    nc = _module_cache[key]

    T = len(tile_t)
    A_PAD = T * TILE_A
    xs = _build_xs_global(features, per_core, A_PAD)

    rep = lambda a: np.concatenate([a] * N_CORES, axis=0)
    global_map = {
        "xs": xs,
        "w1h": rep(w1h),
        "w2h": rep(w2h),
        "wo_h": rep(wo_h),
        "ones8": np.ones((N_CORES * 128, 32), dtype=ml_dtypes.float8_e4m3),
    }
    res = _run_global(nc, global_map)

    ev = res["ev_out"]  # [N_CORES, T//2, 2(row v/e), 2(slot), TILE_A]
    v = ev[:, :, 0].reshape(N_CORES, A_PAD).astype(np.float64)
    e = ev[:, :, 1].reshape(N_CORES, A_PAD).astype(np.float64)
    inv = 1.0 / np.sqrt(np.maximum(v, 0.0) / H2 + LN_EPS)
    ea = e * inv
    energies = np.zeros(NUM_STRUCTS, dtype=np.float64)
    for c in range(N_CORES):
        valid = per_core[c]["valid"]
        perm = per_core[c]["perm"]
        energies += np.bincount(
            batch[perm[valid]], weights=ea[c][valid], minlength=NUM_STRUCTS
        )
    comp = np.bincount(
        batch, weights=comp_w[0].astype(np.float64)[numbers], minlength=NUM_STRUCTS
    )
    return (energies + comp).reshape(NUM_STRUCTS, 1).astype(np.float32)


def kernel(**inputs):
    features = np.asarray(inputs["features"], dtype=np.float32)
    W1 = np.asarray(inputs["W1"], dtype=np.float32)
    W2 = np.asarray(inputs["W2"], dtype=np.float32)
    Wout = np.asarray(inputs["Wout"], dtype=np.float32)
    g1 = np.asarray(inputs["g1"], dtype=np.float32)
    b1 = np.asarray(inputs["b1"], dtype=np.float32)
    g2 = np.asarray(inputs["g2"], dtype=np.float32)
    b2 = np.asarray(inputs["b2"], dtype=np.float32)
    comp_w = np.asarray(inputs["comp_w"], dtype=np.float32)
    numbers = np.asarray(inputs["numbers"]).astype(np.int64)
    batch = np.asarray(inputs["batch"]).astype(np.int64)

    fast_ok = (
        features.shape == (N_ATOMS, N_FEAT)
        and W1.shape == (N_TYPES, N_FEAT, H1)
        and np.all(g1 == 1.0) and np.all(b1 == 0.0)
        and np.all(g2 == 1.0) and np.all(b2 == 0.0)
    )
    if fast_ok:
        try:
            return _device_run(features, W1, W2, Wout, comp_w, numbers, batch)
        except Exception:
            import traceback

            traceback.print_exc()
    return _numpy_reference(
        features, W1, W2, Wout, g1, b1, g2, b2, comp_w, numbers, batch
    )



# revision 39
# speedup vs baseline: 1.1646x; 1.1646x over previous
"""Trainium2 Bass kernel for nn_BPPSModel (type-routed atom MLP + segment pooling).

Strategy (v3):
- Atoms sharded contiguously across 8 cores (50000 each). The host sorts each
  core's atoms by type, pads each type run to a 512 multiple, and lays the
  features out transposed [128 part, 4 kchunk, A_PAD atoms] in bf16 during the
  fp32->bf16 conversion pass. The device then streams tiles with plain
  sequential DMA - no gpsimd gather.
- LayerNorm folding (host): W1/W2 centered over their output dim absorbs the
  mean subtraction; LN scale-invariance (g=1, b=0) lets layer-1's inverse
  sigma cancel inside layer-2's LN; layer-2's inverse sigma is applied on the
  host from a device-computed sum-of-squares v = sum(z2c^2).
- Everything runs in plain bf16 (tolerance 2e-2; measured ~1.1e-3): per tile
  of 512 atoms, 8 L1 matmuls, relu (scalar), 4 L2 matmuls, relu (scalar),
  square (vector), then M=1 reduce-matmuls for e' = wout.relu(z2c) and
  v = ones.z2c^2. Host applies e = e' * rsqrt(v/256 + eps) and pools with
  per-structure bincounts summed across cores.
- Custom PJRT runner: inputs are built directly in the concatenated global
  layout run_bass_via_pjrt would otherwise np.concatenate per call (saves a
  410MB host copy per call).
"""

import numpy as np
import ml_dtypes

N_ATOMS = 400000
N_FEAT = 512
H1 = 256
H2 = 256
N_TYPES = 4
NUM_STRUCTS = 4096
LN_EPS = 1e-5
N_CORES = 8
ATOMS_PER_CORE = N_ATOMS // N_CORES
TILE_A = 512  # atoms per tile (free dim)
KF = N_FEAT // 128  # 4
K2 = H1 // 128  # 2
O1 = H1 // 128  # 2
O2 = H2 // 128  # 2

_module_cache = {}
_runner_cache = {}


def _numpy_reference(features, W1, W2, Wout, g1, b1, g2, b2, comp_w, numbers, batch):
    x = features.astype(np.float32)
    t = numbers.astype(np.int64)

    def linmap(h, W):
        out = np.zeros((h.shape[0], W.shape[2]), dtype=np.float32)
        for ty in range(W.shape[0]):
            m = t == ty
            out[m] = h[m] @ W[ty]
        return out

    def ln(h, g, b):
        mu = h.mean(axis=-1, keepdims=True)
        var = h.var(axis=-1, keepdims=True)
        return (h - mu) / np.sqrt(var + LN_EPS) * g + b

    h = np.maximum(ln(linmap(x, W1), g1, b1), 0.0)
    h = np.maximum(ln(linmap(h, W2), g2, b2), 0.0)
    atom_e = linmap(h, Wout)[:, 0]
    energies = np.bincount(batch.astype(np.int64), weights=atom_e, minlength=NUM_STRUCTS)
    onehot_w = comp_w[0].astype(np.float64)[t]
    comp = np.bincount(batch.astype(np.int64), weights=onehot_w, minlength=NUM_STRUCTS)
    return (energies + comp).reshape(NUM_STRUCTS, 1).astype(np.float32)


def _build_schedule(numbers):
    """Common type-tile schedule + per-core sorted atom permutation.

    Returns (tile_t, per_core) where tile_t[j] is the type of tile j (all
    tiles TILE_A wide) and per_core[c] has perm (padded global atom ids,
    [T*TILE_A]) and valid ([T*TILE_A] bool)."""
    numbers = numbers.astype(np.int64)
    counts = np.zeros((N_CORES, N_TYPES), dtype=np.int64)
    orders = []
    for c in range(N_CORES):
        nb = numbers[c * ATOMS_PER_CORE : (c + 1) * ATOMS_PER_CORE]
        orders.append(np.argsort(nb, kind="stable"))
        counts[c] = np.bincount(nb, minlength=N_TYPES)
    tiles_per_type = [
        int(np.ceil(counts[:, t].max() / TILE_A)) for t in range(N_TYPES)
    ]
    tile_t = []
    for t in range(N_TYPES):
        tile_t.extend([t] * tiles_per_type[t])
    T = len(tile_t)
    A_PAD = T * TILE_A

    per_core = []
    for c in range(N_CORES):
        perm = np.zeros(A_PAD, dtype=np.int64)
        valid = np.zeros(A_PAD, dtype=bool)
        base = c * ATOMS_PER_CORE
        off = 0  # within this core's sorted order
        pos = 0  # within the padded layout
        for t in range(N_TYPES):
            cnt = int(counts[c, t])
            run = base + orders[c][off : off + cnt]
            off += cnt
            width = tiles_per_type[t] * TILE_A
            perm[pos : pos + cnt] = run
            valid[pos : pos + cnt] = True
            if cnt < width:
                # padding lanes: repeat a real atom id so gathered data is
                # defined; masked out on the host afterwards
                perm[pos + cnt : pos + width] = run[-1] if cnt else base
            pos += width
        per_core.append(dict(perm=perm, valid=valid))
    return tile_t, per_core


def _build_module(tile_t):
    import concourse.tile as tile
    from concourse import bacc, mybir

    F32 = mybir.dt.float32
    BF16 = mybir.dt.bfloat16
    F8 = mybir.dt.float8e4
    AF = mybir.ActivationFunctionType
    DR = mybir.MatmulPerfMode.DoubleRow

    T = len(tile_t)
    A_PAD = T * TILE_A
    nc = bacc.Bacc(
        "TRN2", target_bir_lowering=False, debug=False, num_devices=N_CORES,
        enable_asserts=False,
    )
    xs_in = nc.dram_tensor("xs", [128, KF, A_PAD], F8, kind="ExternalInput")
    w1h_in = nc.dram_tensor("w1h", [N_TYPES, N_FEAT, H1], F8, kind="ExternalInput")
    w2h_in = nc.dram_tensor("w2h", [N_TYPES, H1, H2], BF16, kind="ExternalInput")
    woh_in = nc.dram_tensor("wo_h", [N_TYPES, H2], BF16, kind="ExternalInput")
    ones_in = nc.dram_tensor("ones8", [128, 2], F8, kind="ExternalInput")
    ev_out = nc.dram_tensor("ev_out", [T, 2, TILE_A], F32, kind="ExternalOutput")

    with tile.TileContext(nc) as tc:
        with (
            tc.tile_pool(name="const", bufs=1) as cp,
            tc.tile_pool(name="work", bufs=3) as wp,
            tc.tile_pool(name="gat", bufs=6) as gp,
            tc.tile_pool(name="ps1", bufs=2, space="PSUM") as ps1,
            tc.tile_pool(name="ps2", bufs=2, space="PSUM") as ps2,
        ):
            w1h = cp.tile([128, N_TYPES, KF, O1, 128], F8)
            nc.sync.dma_start(
                w1h[:], w1h_in.ap().rearrange("t (k p) (o q) -> p t k o q", p=128, q=128)
            )
            w2h = cp.tile([128, N_TYPES, K2, O2, 128], BF16)
            nc.sync.dma_start(
                w2h[:], w2h_in.ap().rearrange("t (k p) (o q) -> p t k o q", p=128, q=128)
            )
            wofh = cp.tile([128, N_TYPES, K2, 1], BF16)
            nc.sync.dma_start(
                wofh[:], woh_in.ap().rearrange("t (k p) -> p t k", p=128).rearrange("p t k -> p t k ()")
            )
            ones8 = cp.tile([128, 2, 1], F8)
            nc.sync.dma_start(ones8[:], ones_in.ap().rearrange("p k -> p k ()"))

            for j, t in enumerate(tile_t):
                gh = gp.tile([128, KF, TILE_A], F8, tag="gh")
                nc.sync.dma_start(
                    gh[:], xs_in.ap()[:, :, j * TILE_A : (j + 1) * TILE_A]
                )

                # layer 1 in fp8 DoubleRow: each matmul consumes two k-chunks
                # (virtual K=256), halving the instruction count
                z1 = ps1.tile([128, O1, TILE_A], F32, tag="z1")
                for o in range(O1):
                    for k in range(0, KF, 2):
                        nc.tensor.matmul(
                            z1[:, o], w1h[:, t, k : k + 2, o], gh[:, k : k + 2],
                            start=(k == 0), stop=(k == KF - 2),
                            perf_mode=DR,
                        )

                # relu(z1) split across engines: the scalar engine (lighter
                # loaded: Square + ev copy) takes the first 64 atom columns,
                # the vector engine (relu(z2) too) the rest - balances
                # measured per-tile busy (Act ~2.0us vs DVE ~2.5us).
                r1 = wp.tile([128, O1, TILE_A], BF16, tag="r1")
                nc.scalar.activation(r1[:, :, :64], z1[:, :, :64], AF.Relu)
                nc.vector.tensor_scalar_max(r1[:, :, 64:], z1[:, :, 64:], 0.0)

                z2 = ps2.tile([128, O2, TILE_A], F32, tag="z2")
                for o in range(O2):
                    for k in range(K2):
                        nc.tensor.matmul(
                            z2[:, o], w2h[:, t, k, o], r1[:, k],
                            start=(k == 0), stop=(k == K2 - 1),
                        )

                r2 = wp.tile([128, O2, TILE_A], BF16, tag="r2")
                sq = wp.tile([128, O2, TILE_A], F8, tag="sq")
                # balance elementwise work: vector does r1 + half of r2,
                # scalar does sq + the other half of r2
                nc.vector.tensor_scalar_max(r2[:], z2[:], 0.0)
                nc.scalar.activation(sq[:], z2[:], AF.Square)

                # e/v reductions accumulate into z2's PSUM tile (partitions 0
                # and 32 of the o=0 bank) after r2/sq have consumed z2 - this
                # fits the whole pipeline in the 8 PSUM banks with ps1 and ps2
                # both double-buffered.
                e_ps = z2[0:1, 0]
                v_ps = z2[32:33, 0]
                for k in range(K2):
                    nc.tensor.matmul(
                        e_ps, wofh[:, t, k], r2[:, k],
                        start=(k == 0), stop=(k == K2 - 1),
                        tile_position=(0, 0),
                    )
                for k in range(O2):
                    nc.tensor.matmul(
                        v_ps, ones8[:, k], sq[:, k],
                        start=(k == 0), stop=(k == O2 - 1),
                        tile_position=(0, 32),
                    )
                # one staging copy covers both e (partition 0) and v
                # (partition 32); cost is free-dim cycles, not partitions.
                # One strided-partition DMA ships both rows.
                tmp_ev = wp.tile([33, TILE_A], F32, tag="tmp_ev")
                nc.scalar.copy(tmp_ev[:], z2[0:33, 0])
                nc.sync.dma_start(ev_out.ap()[j], tmp_ev[0:33:32])

    nc.compile()
    return nc


def _get_runner(nc):
    """Build (once per module) a jitted shard_map runner that takes inputs
    already concatenated along axis 0 - the layout run_bass_via_pjrt builds
    with np.concatenate on every call."""
    key = id(nc)
    if key in _runner_cache:
        return _runner_cache[key]

    import jax
    from jax.experimental.shard_map import shard_map
    from jax.sharding import Mesh, PartitionSpec
    from concourse import bass2jax, mybir

    bass2jax.install_neuronx_cc_hook()

    partition_name = nc.partition_id_tensor.name if nc.partition_id_tensor else None
    in_names = []
    out_names = []
    out_avals = []
    out_shapes = []
    for alloc in nc.m.functions[0].allocations:
        if not isinstance(alloc, mybir.MemoryLocationSet):
            continue
        name = alloc.memorylocations[0].name
        if alloc.kind == "ExternalInput":
            if name != partition_name:
                in_names.append(name)
        elif alloc.kind == "ExternalOutput":
            shape = tuple(alloc.tensor_shape)
            dtype = mybir.dt.np(alloc.dtype)
            out_avals.append(jax.core.ShapedArray(shape, dtype))
            out_names.append(name)
            out_shapes.append((shape, dtype))
    n_params = len(in_names)
    n_outs = len(out_names)
    all_in_names = list(in_names) + list(out_names)
    if partition_name is not None:
        all_in_names.append(partition_name)
    donate = tuple(range(n_params, n_params + n_outs))

    def _body(*args):
        operands = list(args)
        if partition_name is not None:
            operands.append(bass2jax.partition_id_tensor())
        outs = bass2jax._bass_exec_p.bind(
            *operands,
            out_avals=tuple(out_avals),
            in_names=tuple(all_in_names),
            out_names=tuple(out_names),
            lowering_input_output_aliases=(),
            sim_require_finite=True,
            sim_require_nnan=True,
            nc=nc,
        )
        return tuple(outs)

    devices = jax.devices()[:N_CORES]
    mesh = Mesh(np.asarray(devices), ("core",))
    in_specs = (PartitionSpec("core"),) * (n_params + n_outs)
    out_specs = (PartitionSpec("core"),) * n_outs
    sharded = jax.jit(
        shard_map(
            _body, mesh=mesh, in_specs=in_specs, out_specs=out_specs,
            check_rep=False,
        ),
        donate_argnums=donate,
        keep_unused=True,
    )
    runner = (sharded, in_names, out_names, out_shapes)
    _runner_cache[key] = runner
    return runner


def _run_global(nc, global_map):
    """Run the SPMD module; global_map maps input name -> globally
    concatenated array [N_CORES*d0, ...]. Returns {name: [N_CORES, d0, ...]}."""
    sharded, in_names, out_names, out_shapes = _get_runner(nc)
    ins = [np.asarray(global_map[name]) for name in in_names]
    zeros = [
        np.zeros((N_CORES * s[0], *s[1:]), dt) for (s, dt) in out_shapes
    ]
    outs = sharded(*ins, *zeros)
    return {
        name: np.asarray(arr).reshape(N_CORES, *shape)
        for name, arr, (shape, _) in zip(out_names, outs, out_shapes)
    }


def _f8_twiddle_bits32(rows_f32):
    """f32 -> float8_e4m3 bits (round-half-up, flush-to-zero subnormals).
    Vectorized uint ops; ~20x faster than ml_dtypes' generic cast loop."""
    u = rows_f32.view(np.uint32)
    s = ((u >> 24) & np.uint32(0x80)).astype(np.uint8)
    # round mantissa 23->3 bits, carry naturally into the exponent
    t = ((u & np.uint32(0x7FFFFFFF)) + np.uint32(0x00080000)) >> 20
    t = t.astype(np.int32)
    t -= 960  # rebias exponent 127 -> 7
    np.clip(t, 0, 127, out=t)
    return s | t.astype(np.uint8)


def _build_xs_global(features, per_core, A_PAD):
    """fp32 features -> fp8e4m3, type-sorted, transposed [128, KF, A] per
    core, all cores stacked -> [N_CORES*128, KF, A_PAD]."""
    xs = np.empty((N_CORES * 128, KF, A_PAD), dtype=np.uint8)
    CS = 1024
    for c in range(N_CORES):
        perm = per_core[c]["perm"]
        view = xs[c * 128 : (c + 1) * 128]
        for a0 in range(0, A_PAD, CS):
            n = min(CS, A_PAD - a0)
            rows = _f8_twiddle_bits32(features[perm[a0 : a0 + n]])
            view[:, :, a0 : a0 + n] = rows.reshape(n, KF, 128).transpose(2, 1, 0)
    return xs.view(ml_dtypes.float8_e4m3)


def _device_run(features, W1, W2, Wout, comp_w, numbers, batch):
    W1c = W1 - W1.mean(axis=2, keepdims=True)
    W2c = W2 - W2.mean(axis=2, keepdims=True)
    w1h = W1c.astype(ml_dtypes.float8_e4m3)
    w2h = W2c.astype(ml_dtypes.bfloat16)
    wo_h = np.ascontiguousarray(Wout[:, :, 0]).astype(ml_dtypes.bfloat16)

    tile_t, per_core = _build_schedule(numbers)
    key = tuple(tile_t)
    if key not in _module_cache:
        _module_cache[key] = _build_module(tile_t)
    nc = _module_cache[key]

    T = len(tile_t)
    A_PAD = T * TILE_A
    xs = _build_xs_global(features, per_core, A_PAD)

    rep = lambda a: np.concatenate([a] * N_CORES, axis=0)
    global_map = {
        "xs": xs,
        "w1h": rep(w1h),
        "w2h": rep(w2h),
        "wo_h": rep(wo_h),
        "ones8": np.ones((N_CORES * 128, 2), dtype=ml_dtypes.float8_e4m3),
    }
    res = _run_global(nc, global_map)

    ev = res["ev_out"]  # [N_CORES, T, 2, TILE_A]
    e = ev[:, :, 0].reshape(N_CORES, A_PAD).astype(np.float64)
    v = ev[:, :, 1].reshape(N_CORES, A_PAD).astype(np.float64)
    inv = 1.0 / np.sqrt(np.maximum(v, 0.0) / H2 + LN_EPS)
    ea = e * inv
    energies = np.zeros(NUM_STRUCTS, dtype=np.float64)
    for c in range(N_CORES):
        valid = per_core[c]["valid"]
        perm = per_core[c]["perm"]
        energies += np.bincount(
            batch[perm[valid]], weights=ea[c][valid], minlength=NUM_STRUCTS
        )
    comp = np.bincount(
        batch, weights=comp_w[0].astype(np.float64)[numbers], minlength=NUM_STRUCTS
    )
    return (energies + comp).reshape(NUM_STRUCTS, 1).astype(np.float32)


def kernel(**inputs):
    features = np.asarray(inputs["features"], dtype=np.float32)
    W1 = np.asarray(inputs["W1"], dtype=np.float32)
    W2 = np.asarray(inputs["W2"], dtype=np.float32)
    Wout = np.asarray(inputs["Wout"], dtype=np.float32)
    g1 = np.asarray(inputs["g1"], dtype=np.float32)
    b1 = np.asarray(inputs["b1"], dtype=np.float32)
    g2 = np.asarray(inputs["g2"], dtype=np.float32)
    b2 = np.asarray(inputs["b2"], dtype=np.float32)
    comp_w = np.asarray(inputs["comp_w"], dtype=np.float32)
    numbers = np.asarray(inputs["numbers"]).astype(np.int64)
    batch = np.asarray(inputs["batch"]).astype(np.int64)

    fast_ok = (
        features.shape == (N_ATOMS, N_FEAT)
        and W1.shape == (N_TYPES, N_FEAT, H1)
        and np.all(g1 == 1.0) and np.all(b1 == 0.0)
        and np.all(g2 == 1.0) and np.all(b2 == 0.0)
    )
    if fast_ok:
        try:
            return _device_run(features, W1, W2, Wout, comp_w, numbers, batch)
        except Exception:
            import traceback

            traceback.print_exc()
    return _numpy_reference(
        features, W1, W2, Wout, g1, b1, g2, b2, comp_w, numbers, batch
    )



# revision 40
# speedup vs baseline: 1.2582x; 1.0803x over previous
"""Trainium2 Bass kernel for nn_BPPSModel (type-routed atom MLP + segment pooling).

Strategy (v3):
- Atoms sharded contiguously across 8 cores (50000 each). The host sorts each
  core's atoms by type, pads each type run to a 512 multiple, and lays the
  features out transposed [128 part, 4 kchunk, A_PAD atoms] in bf16 during the
  fp32->bf16 conversion pass. The device then streams tiles with plain
  sequential DMA - no gpsimd gather.
- LayerNorm folding (host): W1/W2 centered over their output dim absorbs the
  mean subtraction; LN scale-invariance (g=1, b=0) lets layer-1's inverse
  sigma cancel inside layer-2's LN; layer-2's inverse sigma is applied on the
  host from a device-computed sum-of-squares v = sum(z2c^2).
- Everything runs in plain bf16 (tolerance 2e-2; measured ~1.1e-3): per tile
  of 512 atoms, 8 L1 matmuls, relu (scalar), 4 L2 matmuls, relu (scalar),
  square (vector), then M=1 reduce-matmuls for e' = wout.relu(z2c) and
  v = ones.z2c^2. Host applies e = e' * rsqrt(v/256 + eps) and pools with
  per-structure bincounts summed across cores.
- Custom PJRT runner: inputs are built directly in the concatenated global
  layout run_bass_via_pjrt would otherwise np.concatenate per call (saves a
  410MB host copy per call).
"""

import numpy as np
import ml_dtypes

N_ATOMS = 400000
N_FEAT = 512
H1 = 256
H2 = 256
N_TYPES = 4
NUM_STRUCTS = 4096
LN_EPS = 1e-5
N_CORES = 8
ATOMS_PER_CORE = N_ATOMS // N_CORES
TILE_A = 512  # atoms per tile (free dim)
KF = N_FEAT // 128  # 4
K2 = H1 // 128  # 2
O1 = H1 // 128  # 2
O2 = H2 // 128  # 2

_module_cache = {}
_runner_cache = {}


def _numpy_reference(features, W1, W2, Wout, g1, b1, g2, b2, comp_w, numbers, batch):
    x = features.astype(np.float32)
    t = numbers.astype(np.int64)

    def linmap(h, W):
        out = np.zeros((h.shape[0], W.shape[2]), dtype=np.float32)
        for ty in range(W.shape[0]):
            m = t == ty
            out[m] = h[m] @ W[ty]
        return out

    def ln(h, g, b):
        mu = h.mean(axis=-1, keepdims=True)
        var = h.var(axis=-1, keepdims=True)
        return (h - mu) / np.sqrt(var + LN_EPS) * g + b

    h = np.maximum(ln(linmap(x, W1), g1, b1), 0.0)
    h = np.maximum(ln(linmap(h, W2), g2, b2), 0.0)
    atom_e = linmap(h, Wout)[:, 0]
    energies = np.bincount(batch.astype(np.int64), weights=atom_e, minlength=NUM_STRUCTS)
    onehot_w = comp_w[0].astype(np.float64)[t]
    comp = np.bincount(batch.astype(np.int64), weights=onehot_w, minlength=NUM_STRUCTS)
    return (energies + comp).reshape(NUM_STRUCTS, 1).astype(np.float32)


def _build_schedule(numbers):
    """Common type-tile schedule + per-core sorted atom permutation.

    Returns (tile_t, per_core) where tile_t[j] is the type of tile j (all
    tiles TILE_A wide) and per_core[c] has perm (padded global atom ids,
    [T*TILE_A]) and valid ([T*TILE_A] bool)."""
    numbers = numbers.astype(np.int64)
    counts = np.zeros((N_CORES, N_TYPES), dtype=np.int64)
    orders = []
    for c in range(N_CORES):
        nb = numbers[c * ATOMS_PER_CORE : (c + 1) * ATOMS_PER_CORE]
        orders.append(np.argsort(nb, kind="stable"))
        counts[c] = np.bincount(nb, minlength=N_TYPES)
    tiles_per_type = [
        int(np.ceil(counts[:, t].max() / TILE_A)) for t in range(N_TYPES)
    ]
    tile_t = []
    for t in range(N_TYPES):
        tile_t.extend([t] * tiles_per_type[t])
    T = len(tile_t)
    A_PAD = T * TILE_A

    per_core = []
    for c in range(N_CORES):
        perm = np.zeros(A_PAD, dtype=np.int64)
        valid = np.zeros(A_PAD, dtype=bool)
        base = c * ATOMS_PER_CORE
        off = 0  # within this core's sorted order
        pos = 0  # within the padded layout
        for t in range(N_TYPES):
            cnt = int(counts[c, t])
            run = base + orders[c][off : off + cnt]
            off += cnt
            width = tiles_per_type[t] * TILE_A
            perm[pos : pos + cnt] = run
            valid[pos : pos + cnt] = True
            if cnt < width:
                # padding lanes: repeat a real atom id so gathered data is
                # defined; masked out on the host afterwards
                perm[pos + cnt : pos + width] = run[-1] if cnt else base
            pos += width
        per_core.append(dict(perm=perm, valid=valid))
    return tile_t, per_core


def _build_module(tile_t):
    import concourse.tile as tile
    from concourse import bacc, mybir

    F32 = mybir.dt.float32
    BF16 = mybir.dt.bfloat16
    F8 = mybir.dt.float8e4
    AF = mybir.ActivationFunctionType
    DR = mybir.MatmulPerfMode.DoubleRow

    T = len(tile_t)
    A_PAD = T * TILE_A
    nc = bacc.Bacc(
        "TRN2", target_bir_lowering=False, debug=False, num_devices=N_CORES,
        enable_asserts=False,
    )
    xs_in = nc.dram_tensor("xs", [128, KF, A_PAD], F8, kind="ExternalInput")
    w1h_in = nc.dram_tensor("w1h", [N_TYPES, N_FEAT, H1], F8, kind="ExternalInput")
    w2h_in = nc.dram_tensor("w2h", [N_TYPES, H1, H2], BF16, kind="ExternalInput")
    woh_in = nc.dram_tensor("wo_h", [N_TYPES, H2], BF16, kind="ExternalInput")
    ones_in = nc.dram_tensor("ones8", [128, 2], F8, kind="ExternalInput")
    ev_out = nc.dram_tensor("ev_out", [T, 2, TILE_A], F32, kind="ExternalOutput")

    with tile.TileContext(nc) as tc:
        with (
            tc.tile_pool(name="const", bufs=1) as cp,
            tc.tile_pool(name="work", bufs=3) as wp,
            tc.tile_pool(name="gat", bufs=6) as gp,
            tc.tile_pool(name="ps1", bufs=2, space="PSUM") as ps1,
            tc.tile_pool(name="ps2", bufs=2, space="PSUM") as ps2,
        ):
            w1h = cp.tile([128, N_TYPES, KF, O1, 128], F8)
            nc.sync.dma_start(
                w1h[:], w1h_in.ap().rearrange("t (k p) (o q) -> p t k o q", p=128, q=128)
            )
            w2h = cp.tile([128, N_TYPES, K2, O2, 128], BF16)
            nc.sync.dma_start(
                w2h[:], w2h_in.ap().rearrange("t (k p) (o q) -> p t k o q", p=128, q=128)
            )
            wofh = cp.tile([128, N_TYPES, K2, 1], BF16)
            nc.sync.dma_start(
                wofh[:], woh_in.ap().rearrange("t (k p) -> p t k", p=128).rearrange("p t k -> p t k ()")
            )
            ones8 = cp.tile([128, 2, 1], F8)
            nc.sync.dma_start(ones8[:], ones_in.ap().rearrange("p k -> p k ()"))

            for j, t in enumerate(tile_t):
                gh = gp.tile([128, KF, TILE_A], F8, tag="gh")
                nc.sync.dma_start(
                    gh[:], xs_in.ap()[:, :, j * TILE_A : (j + 1) * TILE_A]
                )

                # layer 1 in fp8 DoubleRow: each matmul consumes two k-chunks
                # (virtual K=256), halving the instruction count
                z1 = ps1.tile([128, O1, TILE_A], F32, tag="z1")
                for o in range(O1):
                    for k in range(0, KF, 2):
                        nc.tensor.matmul(
                            z1[:, o], w1h[:, t, k : k + 2, o], gh[:, k : k + 2],
                            start=(k == 0), stop=(k == KF - 2),
                            perf_mode=DR,
                        )

                r1 = wp.tile([128, O1, TILE_A], BF16, tag="r1")
                nc.vector.tensor_scalar_max(r1[:], z1[:], 0.0)

                z2 = ps2.tile([128, O2, TILE_A], F32, tag="z2")
                for o in range(O2):
                    for k in range(K2):
                        nc.tensor.matmul(
                            z2[:, o], w2h[:, t, k, o], r1[:, k],
                            start=(k == 0), stop=(k == K2 - 1),
                        )

                r2 = wp.tile([128, O2, TILE_A], BF16, tag="r2")
                sq = wp.tile([128, O2, TILE_A], F8, tag="sq")
                # balance elementwise work: vector does r1 + half of r2,
                # scalar does sq + the other half of r2
                nc.vector.tensor_scalar_max(r2[:], z2[:], 0.0)
                nc.scalar.activation(sq[:], z2[:], AF.Square)

                # e/v reductions accumulate into z2's PSUM tile (partitions 0
                # and 32 of the o=0 bank) after r2/sq have consumed z2 - this
                # fits the whole pipeline in the 8 PSUM banks with ps1 and ps2
                # both double-buffered.
                e_ps = z2[0:1, 0]
                v_ps = z2[32:33, 0]
                for k in range(K2):
                    nc.tensor.matmul(
                        e_ps, wofh[:, t, k], r2[:, k],
                        start=(k == 0), stop=(k == K2 - 1),
                        tile_position=(0, 0),
                    )
                for k in range(O2):
                    nc.tensor.matmul(
                        v_ps, ones8[:, k], sq[:, k],
                        start=(k == 0), stop=(k == O2 - 1),
                        tile_position=(0, 32),
                    )
                # one staging copy covers both e (partition 0) and v
                # (partition 32); cost is free-dim cycles, not partitions.
                # One strided-partition DMA ships both rows.
                tmp_ev = wp.tile([33, TILE_A], F32, tag="tmp_ev")
                nc.scalar.copy(tmp_ev[:], z2[0:33, 0])
                nc.sync.dma_start(ev_out.ap()[j], tmp_ev[0:33:32])

    nc.compile()
    return nc


def _get_runner(nc):
    """Build (once per module) a jitted shard_map runner that takes inputs
    already concatenated along axis 0 - the layout run_bass_via_pjrt builds
    with np.concatenate on every call."""
    key = id(nc)
    if key in _runner_cache:
        return _runner_cache[key]

    import jax
    from jax.experimental.shard_map import shard_map
    from jax.sharding import Mesh, PartitionSpec
    from concourse import bass2jax, mybir

    bass2jax.install_neuronx_cc_hook()

    partition_name = nc.partition_id_tensor.name if nc.partition_id_tensor else None
    in_names = []
    out_names = []
    out_avals = []
    out_shapes = []
    for alloc in nc.m.functions[0].allocations:
        if not isinstance(alloc, mybir.MemoryLocationSet):
            continue
        name = alloc.memorylocations[0].name
        if alloc.kind == "ExternalInput":
            if name != partition_name:
                in_names.append(name)
        elif alloc.kind == "ExternalOutput":
            shape = tuple(alloc.tensor_shape)
            dtype = mybir.dt.np(alloc.dtype)
            out_avals.append(jax.core.ShapedArray(shape, dtype))
            out_names.append(name)
            out_shapes.append((shape, dtype))
    n_params = len(in_names)
    n_outs = len(out_names)
    all_in_names = list(in_names) + list(out_names)
    if partition_name is not None:
        all_in_names.append(partition_name)
    donate = tuple(range(n_params, n_params + n_outs))

    def _body(*args):
        operands = list(args)
        if partition_name is not None:
            operands.append(bass2jax.partition_id_tensor())
        outs = bass2jax._bass_exec_p.bind(
            *operands,
            out_avals=tuple(out_avals),
            in_names=tuple(all_in_names),
            out_names=tuple(out_names),
            lowering_input_output_aliases=(),
            sim_require_finite=True,
            sim_require_nnan=True,
            nc=nc,
        )
        return tuple(outs)

    devices = jax.devices()[:N_CORES]
    mesh = Mesh(np.asarray(devices), ("core",))
    in_specs = (PartitionSpec("core"),) * (n_params + n_outs)
    out_specs = (PartitionSpec("core"),) * n_outs
    sharded = jax.jit(
        shard_map(
            _body, mesh=mesh, in_specs=in_specs, out_specs=out_specs,
            check_rep=False,
        ),
        donate_argnums=donate,
        keep_unused=True,
    )
    runner = (sharded, in_names, out_names, out_shapes)
    _runner_cache[key] = runner
    return runner


def _run_global(nc, global_map):
    """Run the SPMD module; global_map maps input name -> globally
    concatenated array [N_CORES*d0, ...]. Returns {name: [N_CORES, d0, ...]}."""
    sharded, in_names, out_names, out_shapes = _get_runner(nc)
    ins = [np.asarray(global_map[name]) for name in in_names]
    zeros = [
        np.zeros((N_CORES * s[0], *s[1:]), dt) for (s, dt) in out_shapes
    ]
    outs = sharded(*ins, *zeros)
    return {
        name: np.asarray(arr).reshape(N_CORES, *shape)
        for name, arr, (shape, _) in zip(out_names, outs, out_shapes)
    }


def _f8_twiddle_bits32(rows_f32):
    """f32 -> float8_e4m3 bits (round-half-up, flush-to-zero subnormals).
    Vectorized uint ops; ~20x faster than ml_dtypes' generic cast loop."""
    u = rows_f32.view(np.uint32)
    s = ((u >> 24) & np.uint32(0x80)).astype(np.uint8)
    # round mantissa 23->3 bits, carry naturally into the exponent
    t = ((u & np.uint32(0x7FFFFFFF)) + np.uint32(0x00080000)) >> 20
    t = t.astype(np.int32)
    t -= 960  # rebias exponent 127 -> 7
    np.clip(t, 0, 127, out=t)
    return s | t.astype(np.uint8)


def _build_xs_global(features, per_core, A_PAD):
    """fp32 features -> fp8e4m3, type-sorted, transposed [128, KF, A] per
    core, all cores stacked -> [N_CORES*128, KF, A_PAD]."""
    xs = np.empty((N_CORES * 128, KF, A_PAD), dtype=np.uint8)
    CS = 1024
    for c in range(N_CORES):
        perm = per_core[c]["perm"]
        view = xs[c * 128 : (c + 1) * 128]
        for a0 in range(0, A_PAD, CS):
            n = min(CS, A_PAD - a0)
            rows = _f8_twiddle_bits32(features[perm[a0 : a0 + n]])
            view[:, :, a0 : a0 + n] = rows.reshape(n, KF, 128).transpose(2, 1, 0)
    return xs.view(ml_dtypes.float8_e4m3)


def _device_run(features, W1, W2, Wout, comp_w, numbers, batch):
    W1c = W1 - W1.mean(axis=2, keepdims=True)
    W2c = W2 - W2.mean(axis=2, keepdims=True)
    w1h = W1c.astype(ml_dtypes.float8_e4m3)
    w2h = W2c.astype(ml_dtypes.bfloat16)
    wo_h = np.ascontiguousarray(Wout[:, :, 0]).astype(ml_dtypes.bfloat16)

    tile_t, per_core = _build_schedule(numbers)
    key = tuple(tile_t)
    if key not in _module_cache:
        _module_cache[key] = _build_module(tile_t)
    nc = _module_cache[key]

    T = len(tile_t)
    A_PAD = T * TILE_A
    xs = _build_xs_global(features, per_core, A_PAD)

    rep = lambda a: np.concatenate([a] * N_CORES, axis=0)
    global_map = {
        "xs": xs,
        "w1h": rep(w1h),
        "w2h": rep(w2h),
        "wo_h": rep(wo_h),
        "ones8": np.ones((N_CORES * 128, 2), dtype=ml_dtypes.float8_e4m3),
    }
    res = _run_global(nc, global_map)

    ev = res["ev_out"]  # [N_CORES, T, 2, TILE_A]
    e = ev[:, :, 0].reshape(N_CORES, A_PAD).astype(np.float64)
    v = ev[:, :, 1].reshape(N_CORES, A_PAD).astype(np.float64)
    inv = 1.0 / np.sqrt(np.maximum(v, 0.0) / H2 + LN_EPS)
    ea = e * inv
    energies = np.zeros(NUM_STRUCTS, dtype=np.float64)
    for c in range(N_CORES):
        valid = per_core[c]["valid"]
        perm = per_core[c]["perm"]
        energies += np.bincount(
            batch[perm[valid]], weights=ea[c][valid], minlength=NUM_STRUCTS
        )
    comp = np.bincount(
        batch, weights=comp_w[0].astype(np.float64)[numbers], minlength=NUM_STRUCTS
    )
    return (energies + comp).reshape(NUM_STRUCTS, 1).astype(np.float32)


def kernel(**inputs):
    features = np.asarray(inputs["features"], dtype=np.float32)
    W1 = np.asarray(inputs["W1"], dtype=np.float32)
    W2 = np.asarray(inputs["W2"], dtype=np.float32)
    Wout = np.asarray(inputs["Wout"], dtype=np.float32)
    g1 = np.asarray(inputs["g1"], dtype=np.float32)
    b1 = np.asarray(inputs["b1"], dtype=np.float32)
    g2 = np.asarray(inputs["g2"], dtype=np.float32)
    b2 = np.asarray(inputs["b2"], dtype=np.float32)
    comp_w = np.asarray(inputs["comp_w"], dtype=np.float32)
    numbers = np.asarray(inputs["numbers"]).astype(np.int64)
    batch = np.asarray(inputs["batch"]).astype(np.int64)

    fast_ok = (
        features.shape == (N_ATOMS, N_FEAT)
        and W1.shape == (N_TYPES, N_FEAT, H1)
        and np.all(g1 == 1.0) and np.all(b1 == 0.0)
        and np.all(g2 == 1.0) and np.all(b2 == 0.0)
    )
    if fast_ok:
        try:
            return _device_run(features, W1, W2, Wout, comp_w, numbers, batch)
        except Exception:
            import traceback

            traceback.print_exc()
    return _numpy_reference(
        features, W1, W2, Wout, g1, b1, g2, b2, comp_w, numbers, batch
    )

